# revision 10
# baseline (speedup 1.0000x reference)
"""BackpropWiSARD embedding-lookup kernel for 8 Trainium2 NeuronCores.

Strategy (data-parallel over batch, table replicated):
  - Host: table (C,F,E) -> (F,E,C) bf16 padded to 128 classes, rows grouped
    4 f's per window (f = t*112 + half*56 + g, f_l = 2*half+t) so each
    dma_gather call addresses a 32768-row window; max index 32767 fits int16.
  - Each core handles B/8 = 64 batch rows.
  - Device per core:
      1. 56 scalar-offset indirect DMAs gather x^T rows by input_order ->
         mapped bits [p=f%112, (t,i), b]  (walrus supports only one dynamic
         offset per partition per indirect DMA).
      2. H3 hash on DVE: masked = mapped * hv[h,i] (int32), XOR-tree over i,
         XOR with t*8192 -> window-local row indices (int16 range).
      3. Index shuffle to dma_gather's int16 16-partition wrap layout via a
         DRAM round trip (8 strided DMAs out, 8 replica loads back).
      4. 56 dma_gather calls (1024 idx each): call g fetches rows of table
         window g into Mc[p=t*64+b, g, h, plo, c].
      5. min over h, binarize (is_ge 0), tree-sum over g -> acc[p, c];
         selection-matrix matmul folds p=t*64+b partition pairs -> psum[b,c];
         affine 2S-F, add bias, DMA out (64,100) per core.
"""

import sys

sys.path.insert(0, "/opt/trn_rl_repo")

import numpy as np
import ml_dtypes

B, C, F, E, H, I = 512, 100, 224, 8192, 4, 28
NB = F * I  # 6272 input bits
NCORES = 8
BP = B // NCORES  # 64 batch rows per core
P1 = 112  # partitions carrying f % 112
T = F // P1  # 2
IP = 32  # i padded to power of two for the XOR tree
CP = 128  # classes padded for 256B gather rows
GW = 4 * E  # 32768-row table window (max idx 32767 fits int16 exactly)
NG = 56  # windows; window g holds f = t*112 + half*56 + g, f_l = 2*half+t

_NC = None


def _build(loop_reps=1):
    import contextlib

    import concourse.bass as bass
    import concourse.mybir as mybir
    import concourse.tile as tile
    from concourse import bacc
    from concourse.library_config import mlp

    dt = mybir.dt
    op = mybir.AluOpType

    nc = bacc.Bacc(
        "TRN2", target_bir_lowering=False, debug=False, num_swdge_queues=4
    )

    tbl = nc.dram_tensor("tbl", (NG * GW, CP), dt.bfloat16, kind="ExternalInput")
    mapd = nc.dram_tensor("mapd", (P1, T * I * BP), dt.int32, kind="ExternalInput")
    hvx = nc.dram_tensor("hvx", (P1, H * IP), dt.int32, kind="ExternalInput")
    rofs = nc.dram_tensor("rofs", (P1, T), dt.int32, kind="ExternalInput")
    sel = nc.dram_tensor("sel", (CP, BP), dt.bfloat16, kind="ExternalInput")
    biasx = nc.dram_tensor("biasx", (BP, C), dt.float32, kind="ExternalInput")
    outd = nc.dram_tensor("out", (BP, C), dt.float32, kind="ExternalOutput")

    with tile.TileContext(nc) as tc:
        nc.gpsimd.load_library(mlp)
        with (
            tc.tile_pool(name="main", bufs=1) as pool,
            tc.tile_pool(name="mc", bufs=4) as mpool,
            tc.tile_pool(name="dram", bufs=1, space="DRAM") as dpool,
            tc.tile_pool(name="psum", bufs=2, space="PSUM") as psum_pool,
            (tc.For_i(0, loop_reps, 1) if loop_reps > 1 else contextlib.nullcontext()),
        ):
            hvx_sb = pool.tile([P1, H, 1, IP, 1], dt.int32)
            nc.sync.dma_start(
                out=hvx_sb[:].rearrange("p h o i z -> p (h o i z)"), in_=hvx.ap()
            )
            bias_sb = pool.tile([BP, C], dt.float32)
            nc.sync.dma_start(out=bias_sb[:], in_=biasx.ap())
            sel_sb = pool.tile([CP, BP], dt.bfloat16)
            nc.sync.dma_start(out=sel_sb[:], in_=sel.ap())
            rowoff = pool.tile([P1, T, 1, 1], dt.int32)
            nc.sync.dma_start(
                out=rowoff[:].rearrange("p t o z -> p (t o z)"), in_=rofs.ap()
            )

            # mapped[p,(t,i),b] = x[b, input_order[f*I+i]],  f = t*P1+p
            # (permutation applied on host; pure data movement)
            mapped = pool.tile([P1, T * I, BP], dt.int32)
            nc.sync.dma_start(
                out=mapped[:].rearrange("p k b -> p (k b)"), in_=mapd.ap()
            )
            mapped4 = mapped[:].rearrange("p (t i) b -> p t i b", t=T)

            msk = pool.tile([P1, T, IP, BP], dt.int32)
            nc.vector.memset(msk[:, :, I:IP, :], 0)
            idxs = []
            for h in range(H):
                hv_h = hvx_sb[:, h, :, 0:I, :].to_broadcast([P1, T, I, BP])
                nc.vector.tensor_tensor(
                    out=msk[:, :, 0:I, :], in0=mapped4, in1=hv_h, op=op.mult
                )
                w = IP
                while w > 1:
                    w //= 2
                    nc.vector.tensor_tensor(
                        out=msk[:, :, 0:w, :],
                        in0=msk[:, :, 0:w, :],
                        in1=msk[:, :, w : 2 * w, :],
                        op=op.bitwise_xor,
                    )
                idx_h = pool.tile([P1, T, 1, BP], dt.int32, tag=f"idx{h}")
                ro = rowoff[:].to_broadcast([P1, T, 1, BP])
                nc.vector.tensor_tensor(
                    out=idx_h[:], in0=msk[:, :, 0:1, :], in1=ro, op=op.bitwise_xor
                )
                idxs.append(idx_h)

            # Shuffle indices into dma_gather's int16 wrap layout:
            # call g uses idx j = h*128 + t*64 + b (dst partition j%128 =
            # t*64+b, dst slot j//128 = h); idx tile position [j%16, j//16]
            # = [b%16, h*8 + t*4 + b//16].
            idx16 = []
            for h in range(H):
                i16 = pool.tile([P1, T, 4, 16], dt.int16, tag=f"i16_{h}")
                nc.vector.tensor_copy(
                    out=i16[:].rearrange("p t bh bl -> p (t bh bl)"),
                    in_=idxs[h][:].rearrange("p t o b -> p (t o b)"),
                )
                idx16.append(i16)
            # idx tile col j//16 = h*16 + f_l*4 + b//16 within window g's
            # 64-col block; partition half p//56 selects f_l = 2*half+t.
            dram_idx = dpool.tile([NG, H, 4, 4, 16], dt.int16)
            for h in range(H):
                for t in range(T):
                    for half in range(2):
                        fl = 2 * half + t
                        src = idx16[h][
                            half * NG : (half + 1) * NG, t : t + 1, :, :
                        ].rearrange("p to bh bl -> p (to bh bl)")
                        dst = dram_idx[:, h : h + 1, fl : fl + 1, :, :].rearrange(
                            "g ho flo bh bl -> g (ho flo bh bl)"
                        )
                        nc.sync.dma_start(out=dst, in_=src)
            idxT = pool.tile([128, NG * H * 4 * 4], dt.int16)
            dflat = dram_idx[:].rearrange("g h fl bh bl -> bl (g h fl bh)")
            for r in range(8):
                nc.sync.dma_start(out=idxT[r * 16 : (r + 1) * 16, :], in_=dflat)

            # Gather: call g fetches 512 rows from table window g into
            # Mc[p=t*64+b, gi, h, c]; then min over h, binarize, sum over g.
            acc = pool.tile([CP, 1, 1, 1, CP], dt.bfloat16)
            nc.vector.memset(acc[:], 0)
            GC = 14
            for g0 in range(0, NG, GC):
                # slot j//128 = h*2 + plo -> Mc[p, gi, h, plo, c]
                Mc = mpool.tile([128, GC, H, 2, CP], dt.bfloat16, tag="Mc")
                for gi in range(GC):
                    g = g0 + gi
                    nc.gpsimd.dma_gather(
                        out_ap=Mc[:, gi, :, :, :].rearrange("p h plo c -> p (h plo) c"),
                        in_ap=tbl.ap()[g * GW : (g + 1) * GW, :],
                        idxs_ap=idxT[:, g * 64 : (g + 1) * 64],
                        num_idxs=1024,
                        num_idxs_reg=1024,
                        elem_size=CP,
                        queue_num=g % 4,
                    )
                nc.vector.tensor_tensor(
                    out=Mc[:, :, 0:1, :, :], in0=Mc[:, :, 0:1, :, :], in1=Mc[:, :, 1:2, :, :], op=op.min
                )
                nc.vector.tensor_tensor(
                    out=Mc[:, :, 2:3, :, :], in0=Mc[:, :, 2:3, :, :], in1=Mc[:, :, 3:4, :, :], op=op.min
                )
                nc.vector.tensor_tensor(
                    out=Mc[:, :, 0:1, :, :], in0=Mc[:, :, 0:1, :, :], in1=Mc[:, :, 2:3, :, :], op=op.min
                )
                nc.vector.tensor_scalar(
                    out=Mc[:, :, 0:1, :, :],
                    in0=Mc[:, :, 0:1, :, :],
                    scalar1=0.0,
                    scalar2=None,
                    op0=op.is_ge,
                )
                # tree-sum the GC {0,1} slabs, then accumulate
                nc.vector.tensor_tensor(
                    out=Mc[:, :, 0:1, 0:1, :],
                    in0=Mc[:, :, 0:1, 0:1, :],
                    in1=Mc[:, :, 0:1, 1:2, :],
                    op=op.add,
                )
                w = GC
                while w > 1:
                    lo = w // 2
                    nc.vector.tensor_tensor(
                        out=Mc[:, 0:lo, 0:1, 0:1, :],
                        in0=Mc[:, 0:lo, 0:1, 0:1, :],
                        in1=Mc[:, lo : 2 * lo, 0:1, 0:1, :],
                        op=op.add,
                    )
                    if w % 2:
                        nc.vector.tensor_tensor(
                            out=Mc[:, 0:1, 0:1, 0:1, :],
                            in0=Mc[:, 0:1, 0:1, 0:1, :],
                            in1=Mc[:, w - 1 : w, 0:1, 0:1, :],
                            op=op.add,
                        )
                    w = lo
                nc.vector.tensor_tensor(
                    out=acc[:], in0=acc[:], in1=Mc[:, 0:1, 0:1, 0:1, :], op=op.add
                )

            # fold p = t*64+b partition pairs: psum[b,c] = sum_p sel[p,b]*acc[p,c]
            S = psum_pool.tile([BP, CP], dt.float32, tag="S")
            nc.tensor.matmul(
                out=S[:],
                lhsT=sel_sb[:],
                rhs=acc[:].rearrange("p o z y c -> p (o z y c)"),
                start=True,
                stop=True,
            )
            res = pool.tile([BP, C], dt.float32)
            nc.vector.tensor_scalar(
                out=res[:],
                in0=S[:, 0:C],
                scalar1=2.0,
                scalar2=float(-F),
                op0=op.mult,
                op1=op.add,
            )
            nc.vector.tensor_tensor(out=res[:], in0=res[:], in1=bias_sb[:], op=op.add)
            nc.sync.dma_start(out=outd.ap(), in_=res[:])

    nc.compile()
    return nc


def get_nc(loop_reps=1):
    global _NC
    if loop_reps != 1:
        return _build(loop_reps)
    if _NC is None:
        _NC = _build()
    return _NC


def prep_in_maps(inputs):
    x_b = np.asarray(inputs["x_b"], dtype=np.int32)
    input_order = np.asarray(inputs["input_order"], dtype=np.int32)
    hash_values = np.asarray(inputs["hash_values"], dtype=np.int32)
    table = np.asarray(inputs["table"], dtype=np.float32)
    bias = np.asarray(inputs["bias"], dtype=np.float32)

    # (C,F,E) -> (F,E,CP) bf16 rows (sign-preserving cast; pad classes with
    # zeros), then group rows so window g holds f = g and f = g+112.
    tp = np.zeros((F, E, CP), dtype=ml_dtypes.bfloat16)
    tp[:, :, :C] = table.transpose(1, 2, 0).astype(ml_dtypes.bfloat16)
    g_ = np.arange(NG)[:, None]
    fl_ = np.arange(4)[None, :]
    fmap = ((fl_ % 2) * P1 + (fl_ // 2) * NG + g_).reshape(-1)  # (56*4,)
    tt = np.ascontiguousarray(tp[fmap]).reshape(NG * GW, CP)

    io_arr = np.ascontiguousarray(
        input_order.reshape(T, P1, I).transpose(1, 0, 2)
    ).reshape(P1, T * I)
    io_flat = io_arr.reshape(-1)

    hvx = np.zeros((P1, H, IP), dtype=np.int32)
    hvx[:, :, :I] = hash_values[None, :, :]
    hvx = hvx.reshape(P1, H * IP)

    # idx value = f_l*E + hash, f_l = 2*(p//56) + t
    half_ = (np.arange(P1, dtype=np.int32) // NG)[:, None]
    rofs = ((2 * half_ + np.arange(T, dtype=np.int32)[None, :]) * E).astype(np.int32)

    selm = np.tile(np.eye(BP, dtype=np.float32), (T, 1)).astype(ml_dtypes.bfloat16)
    selm = np.ascontiguousarray(selm)  # (128, 64)

    biasx = np.ascontiguousarray(np.tile(bias.reshape(1, C), (BP, 1)))

    in_maps = []
    for k in range(NCORES):
        xtk = x_b[k * BP : (k + 1) * BP].T
        mapd = np.ascontiguousarray(xtk[io_flat]).reshape(P1, T * I * BP)
        in_maps.append(
            {
                "tbl": tt,
                "mapd": mapd,
                "hvx": hvx,
                "rofs": rofs,
                "sel": selm,
                "biasx": biasx,
            }
        )
    return in_maps


def kernel(**inputs):
    from concourse.bass_utils import run_bass_kernel_spmd

    nc = get_nc()
    in_maps = prep_in_maps(inputs)
    res = run_bass_kernel_spmd(nc, in_maps, list(range(NCORES)))
    parts = [res.results[k]["out"].reshape(BP, C) for k in range(NCORES)]
    return np.concatenate(parts, axis=0).astype(np.float32)



# revision 15
# speedup vs baseline: 7.3223x; 7.3223x over previous
"""BackpropWiSARD embedding-lookup kernel for 8 Trainium2 NeuronCores, v3.

Data-parallel over batch (64 rows/core), table replicated, bf16 classes.

Key idea vs v2: the hash pipeline computes gather indices DIRECTLY in
dma_gather's wrap layout, eliminating the DRAM index-shuffle round trip
(which dominated the baseline at ~900us of 2-byte scattered descriptors).

dma_gather ucode facts used:
  - call on queue q is processed by Q7 cpu pair (2q, 2q+1), which reads the
    index tile ONLY from partitions [32q, 32q+32) (two 16-row replicas).
  - wrap layout: index j of a call sits at [j%16, j//16] of those rows.

Layouts (per core):
  - partitions p = q*32 + r*16 + b_lo  (q = queue, r = replica duplicate,
    b_lo = b%16).  Window g = gq*4 + q holds f's {fl*56+g} (fl = flh*2+fl2),
    rows fl*8192+e of the 32768-row window; gathers for window g are issued
    on queue q, so partitions of queue q hold exactly its windows' indices.
  - mapped[p, gq, 1, (flh fl2 b_hi), i] int16 = x[b, input_order[f*28+i]]
    (host applies the input_order permutation; pure data movement).
  - hash: msk = mapped * hv[h, i] (broadcast over h), XOR-tree over i=28,
    XOR with fl*8192 -> written to idxT[:, gq*64 + h*16 + flh*8 + fl2*4 + b_hi].
  - call (gq, q): idxs_ap = idxT[:, gq*64:(gq+1)*64] for every q; queue q's
    cpu pair sees its own windows' indices.  j = col*16 + b_lo =>
    Mc partition p_dst = fl2*64 + b, slot j//128 = h*2 + flh.
  - reduce: min over h (slot pairs), binarize (is_ge 0), add flh pair, sum
    windows, accumulate; PE matmul with a (128,64) selection matrix folds the
    fl2 partition pairs; affine 2S-F + bias; out (64,100) f32 per core.
"""

import sys

sys.path.insert(0, "/opt/trn_rl_repo")

import numpy as np
import ml_dtypes

B, C, F, E, H, I = 512, 100, 224, 8192, 4, 28
NB = F * I
NCORES = 8
BP = B // NCORES  # 64
CP = 128
GW = 4 * E  # 32768-row window
NG = 56  # windows; window g holds f = fl*56 + g
NGQ = 14  # window-column groups (g = gq*4 + q)
GRP = 16  # (flh, fl2, b_hi)
CHUNK = 2  # gq per chunk
NCHUNK = NGQ // CHUNK  # 7

_NC = None


def _build(loop_reps=1):
    import contextlib

    import concourse.bass as bass
    import concourse.mybir as mybir
    import concourse.tile as tile
    from concourse import bacc
    from concourse.library_config import mlp

    dt = mybir.dt
    op = mybir.AluOpType

    nc = bacc.Bacc(
        "TRN2", target_bir_lowering=False, debug=False, num_swdge_queues=4
    )

    tbl = nc.dram_tensor("tbl", (NG * GW, CP), dt.bfloat16, kind="ExternalInput")
    mapd = nc.dram_tensor("mapd", (128, NGQ * GRP * I), dt.int16, kind="ExternalInput")
    hvx = nc.dram_tensor("hvx", (128, H * GRP * I), dt.int16, kind="ExternalInput")
    rofs = nc.dram_tensor("rofs", (128, GRP), dt.int16, kind="ExternalInput")
    sel = nc.dram_tensor("sel", (CP, BP), dt.bfloat16, kind="ExternalInput")
    biasx = nc.dram_tensor("biasx", (BP, C), dt.float32, kind="ExternalInput")
    outd = nc.dram_tensor("out", (BP, C), dt.float32, kind="ExternalOutput")

    with tile.TileContext(nc) as tc:
        nc.gpsimd.load_library(mlp)
        with (
            tc.tile_pool(name="main", bufs=1) as pool,
            tc.tile_pool(name="msk", bufs=2) as kpool,
            tc.tile_pool(name="mc", bufs=4) as mpool,
            tc.tile_pool(name="psum", bufs=2, space="PSUM") as psum_pool,
            (tc.For_i(0, loop_reps, 1) if loop_reps > 1 else contextlib.nullcontext()),
        ):
            # [p, gq, 1(h), grp, i]
            mapped = pool.tile([128, NGQ, 1, GRP, I], dt.int16)
            nc.sync.dma_start(
                out=mapped[:].rearrange("p gq o grp i -> p (gq o grp i)"),
                in_=mapd.ap(),
            )
            hvp = pool.tile([128, 1, H, GRP * I], dt.int16)
            nc.sync.dma_start(
                out=hvp[:].rearrange("p o h gi -> p (o h gi)"), in_=hvx.ap()
            )
            rofs_sb = pool.tile([128, 1, 1, GRP], dt.int16)
            nc.sync.dma_start(
                out=rofs_sb[:].rearrange("p o z grp -> p (o z grp)"), in_=rofs.ap()
            )
            bias_sb = pool.tile([BP, C], dt.float32)
            nc.sync.dma_start(out=bias_sb[:], in_=biasx.ap())
            sel_sb = pool.tile([CP, BP], dt.bfloat16)
            nc.sync.dma_start(out=sel_sb[:], in_=sel.ap())

            idxT = pool.tile([128, NGQ, H, GRP], dt.int16)
            acc = pool.tile([CP, 1, 1, CP], dt.bfloat16)
            nc.vector.memset(acc[:], 0)

            for c in range(NCHUNK):
                gqs = slice(c * CHUNK, (c + 1) * CHUNK)
                msk = kpool.tile([128, CHUNK, H, GRP, I], dt.int16, tag="msk")
                nc.vector.tensor_tensor(
                    out=msk[:].rearrange("p gq h grp i -> p gq h (grp i)"),
                    in0=mapped[:, gqs, :, :, :]
                    .rearrange("p gq o grp i -> p gq o (grp i)")
                    .to_broadcast([128, CHUNK, H, GRP * I]),
                    in1=hvp[:].to_broadcast([128, CHUNK, H, GRP * I]),
                    op=op.mult,
                )
                # XOR-reduce over i=28: 14/7/(3,3)+tail
                msk2 = msk[:].rearrange("p gq h grp i -> p (gq h grp) i")
                for (d0, w) in ((14, 14), (7, 7), (3, 3), (1, 1), (2, 1), (6, 1)):
                    nc.vector.tensor_tensor(
                        out=msk2[:, :, 0 : min(w, d0)],
                        in0=msk2[:, :, 0 : min(w, d0)],
                        in1=msk2[:, :, d0 : d0 + w],
                        op=op.bitwise_xor,
                    )
                # idx = msk ^ fl*8192 -> wrap columns
                nc.vector.tensor_tensor(
                    out=idxT[:, gqs, :, :],
                    in0=msk[:, :, :, :, 0:1].rearrange("p gq h grp o -> p gq h (grp o)"),
                    in1=rofs_sb[:].to_broadcast([128, CHUNK, H, GRP]),
                    op=op.bitwise_xor,
                )

                Mc = mpool.tile([128, CHUNK * 4, 2 * H, CP], dt.bfloat16, tag="Mc")
                for gql in range(CHUNK):
                    gq = c * CHUNK + gql
                    for q in range(4):
                        g = gq * 4 + q
                        nc.gpsimd.dma_gather(
                            out_ap=Mc[:, gql * 4 + q, :, :],
                            in_ap=tbl.ap()[g * GW : (g + 1) * GW, :],
                            idxs_ap=idxT[:, gq, :, :].rearrange(
                                "p h grp -> p (h grp)"
                            ),
                            num_idxs=1024,
                            num_idxs_reg=1024,
                            elem_size=CP,
                            queue_num=q,
                        )
                # slots s = h*2 + flh; min over h
                nc.vector.tensor_tensor(
                    out=Mc[:, :, 0:4, :],
                    in0=Mc[:, :, 0:4, :],
                    in1=Mc[:, :, 4:8, :],
                    op=op.min,
                )
                nc.vector.tensor_tensor(
                    out=Mc[:, :, 0:2, :],
                    in0=Mc[:, :, 0:2, :],
                    in1=Mc[:, :, 2:4, :],
                    op=op.min,
                )
                nc.vector.tensor_scalar(
                    out=Mc[:, :, 0:2, :],
                    in0=Mc[:, :, 0:2, :],
                    scalar1=0.0,
                    scalar2=None,
                    op0=op.is_ge,
                )
                nc.vector.tensor_tensor(
                    out=Mc[:, :, 0:1, :],
                    in0=Mc[:, :, 0:1, :],
                    in1=Mc[:, :, 1:2, :],
                    op=op.add,
                )
                # sum the 8 window slabs
                w = CHUNK * 4
                while w > 1:
                    lo = w // 2
                    nc.vector.tensor_tensor(
                        out=Mc[:, 0:lo, 0:1, :],
                        in0=Mc[:, 0:lo, 0:1, :],
                        in1=Mc[:, lo : 2 * lo, 0:1, :],
                        op=op.add,
                    )
                    w = lo
                nc.vector.tensor_tensor(
                    out=acc[:],
                    in0=acc[:],
                    in1=Mc[:, 0:1, 0:1, :],
                    op=op.add,
                )

            S = psum_pool.tile([BP, CP], dt.float32, tag="S")
            nc.tensor.matmul(
                out=S[:],
                lhsT=sel_sb[:],
                rhs=acc[:].rearrange("p o z c -> p (o z c)"),
                start=True,
                stop=True,
            )
            res = pool.tile([BP, C], dt.float32)
            nc.vector.tensor_scalar(
                out=res[:],
                in0=S[:, 0:C],
                scalar1=2.0,
                scalar2=float(-F),
                op0=op.mult,
                op1=op.add,
            )
            nc.vector.tensor_tensor(out=res[:], in0=res[:], in1=bias_sb[:], op=op.add)
            nc.sync.dma_start(out=outd.ap(), in_=res[:])

    nc.compile()
    return nc


def get_nc(loop_reps=1):
    global _NC
    if loop_reps != 1:
        return _build(loop_reps)
    if _NC is None:
        _NC = _build()
    return _NC


def prep_in_maps(inputs):
    x_b = np.asarray(inputs["x_b"], dtype=np.int32)
    input_order = np.asarray(inputs["input_order"], dtype=np.int32)
    hash_values = np.asarray(inputs["hash_values"], dtype=np.int32)
    table = np.asarray(inputs["table"], dtype=np.float32)
    bias = np.asarray(inputs["bias"], dtype=np.float32)

    # (C,F,E) -> rows [(g*4+fl)*8192 + e] = f = fl*56 + g, 128-class bf16 rows
    tp = np.zeros((F, E, CP), dtype=ml_dtypes.bfloat16)
    tp[:, :, :C] = table.transpose(1, 2, 0).astype(ml_dtypes.bfloat16)
    g_ = np.arange(NG)[:, None]
    fl_ = np.arange(4)[None, :]
    fmap = (fl_ * NG + g_).reshape(-1)  # (g,fl) -> f
    tt = np.ascontiguousarray(tp[fmap]).reshape(NG * GW, CP)

    # per-core mapped[p=(q,r,b_lo), gq, grp=(flh,fl2,b_hi), i] =
    #   x[b_hi*16+b_lo, input_order[f*28+i]],  f=(flh*2+fl2)*56 + gq*4 + q
    q_ = np.arange(4)[:, None, None, None, None, None]  # q
    gq_ = np.arange(NGQ)[None, None, None, :, None, None]
    flh_ = np.arange(2)[None, None, None, None, :, None]
    fl2_ = np.arange(2)[None, None, None, None, None, :]
    f_full = (flh_ * 2 + fl2_) * NG + gq_ * 4 + q_  # [4,1,1,14,2,2]
    i_ = np.arange(I)
    bit_idx = f_full[..., None] * I + i_  # [4,1,1,14,2,2,28]
    bit_idx = np.broadcast_to(bit_idx, (4, 2, 16, NGQ, 2, 2, I))
    order_bits = input_order[bit_idx]  # bit position per slot

    hvx = np.ascontiguousarray(
        np.broadcast_to(
            hash_values[None, :, None, :].astype(np.int16), (128, H, GRP, I)
        )
    ).reshape(128, H * GRP * I)

    grp_ = np.arange(GRP)
    rofs = np.ascontiguousarray(
        np.broadcast_to(((grp_ // 4) * E).astype(np.int16)[None, :], (128, GRP))
    )

    selm = np.ascontiguousarray(
        np.tile(np.eye(BP, dtype=np.float32), (2, 1)).astype(ml_dtypes.bfloat16)
    )
    biasx = np.ascontiguousarray(np.tile(bias.reshape(1, C), (BP, 1)))

    b_lo_ = np.arange(16)[None, None, :, None, None, None]
    b_hi_ = np.arange(4)
    in_maps = []
    for k in range(NCORES):
        xk = x_b[k * BP : (k + 1) * BP]  # (64, 6272)
        b_full = (b_hi_[None, None, None, None, None, None, :] * 16 + b_lo_[..., None])
        # shape [4(q),2(r),16(b_lo),14,2,2,4(b_hi)] -> batch index
        b_full = np.broadcast_to(b_full, (4, 2, 16, NGQ, 2, 2, 4))
        # mapped value = xk[b, order_bits[...]] with grp=(flh,fl2,b_hi)
        ob = np.broadcast_to(
            order_bits[:, :, :, :, :, :, None, :], (4, 2, 16, NGQ, 2, 2, 4, I)
        )
        bf = np.broadcast_to(b_full[..., None], (4, 2, 16, NGQ, 2, 2, 4, I))
        mapd = xk[bf, ob].astype(np.int16)  # [q,r,b_lo,gq,flh,fl2,b_hi,i]
        mapd = np.ascontiguousarray(mapd.reshape(128, NGQ * GRP * I))
        in_maps.append(
            {
                "tbl": tt,
                "mapd": mapd,
                "hvx": hvx,
                "rofs": rofs,
                "sel": selm,
                "biasx": biasx,
            }
        )
    return in_maps


def kernel(**inputs):
    from concourse.bass_utils import run_bass_kernel_spmd

    nc = get_nc()
    in_maps = prep_in_maps(inputs)
    res = run_bass_kernel_spmd(nc, in_maps, list(range(NCORES)))
    parts = [res.results[k]["out"].reshape(BP, C) for k in range(NCORES)]
    return np.concatenate(parts, axis=0).astype(np.float32)
